# revision 1
# baseline (speedup 1.0000x reference)
"""Trainium2 Bass kernel for nn_DataWindowLoss: mean(|box5x5(x) - box5x5(y)|).

Math: the 5x5 uniform box filter (padding=4) is linear and separable, so
    box(x) - box(y) = box(x - y) = A @ (x - y) @ A^T   (per image)
where A is the [516, 512] banded matrix with A[o, r] = 1/25 for o-4 <= r <= o.
We use band entries of 1.0 and divide by 25 at the very end on the host.

Per image (on-chip, all contraction via the TensorEngine):
  pass1: VT[w, o] = sum_r x[r, w] * AT[r, o] - sum_r y[r, w] * AT[r, o]
         (lhsT = x/y tiles, rhs = +/- banded constants, accumulated in PSUM)
  pass2: HT[c, o] = sum_w B[w, c] * VT[w, o]   (lhsT = banded constants)
  then sum |HT| with fused abs+reduce on the VectorEngine.

Sharding: pure data parallel - 8 images per core on 8 NeuronCores; each core
emits a [128, 1] fp32 partial-sum vector; the host reduces and normalizes.
"""

import sys

sys.path.insert(0, "/opt/trn_rl_repo")

import numpy as np

import concourse.mybir as mybir
import concourse.tile as tile
from concourse import bacc
from concourse.bass_utils import run_bass_kernel_spmd

N_CORES = 8
IMG_PER_CORE = 8
P = 128          # partitions
HW = 512         # image height/width
KT = 4           # r-tiles / w-tiles per image
OUT = 516        # output spatial size (512 + 2*4 - 5 + 1)
F16 = mybir.dt.float16
F32 = mybir.dt.float32


def _make_band_consts(nc, pool):
    """Banded +/-1 constants in fp16.

    bandP/bandN [128, 132]: band[p, j] = +/-1 iff p <= j <= p+4
    bandL/bandLn [128, 128]: corner[p, q] = +/-1 iff p - q >= 124
    """
    bandP = pool.tile([P, 132], F16)
    bandN = pool.tile([P, 132], F16)
    bandL = pool.tile([P, 128], F16)
    bandLn = pool.tile([P, 128], F16)
    for t, val in ((bandP, 1.0), (bandN, -1.0)):
        nc.gpsimd.memset(t, val)
        # keep iff j - p >= 0
        nc.gpsimd.affine_select(
            out=t, in_=t, compare_op=mybir.AluOpType.is_ge, fill=0.0,
            base=0, pattern=[[1, 132]], channel_multiplier=-1)
        # keep iff p + 4 - j >= 0
        nc.gpsimd.affine_select(
            out=t, in_=t, compare_op=mybir.AluOpType.is_ge, fill=0.0,
            base=4, pattern=[[-1, 132]], channel_multiplier=1)
    for t, val in ((bandL, 1.0), (bandLn, -1.0)):
        nc.gpsimd.memset(t, val)
        # keep iff p - q - 124 >= 0
        nc.gpsimd.affine_select(
            out=t, in_=t, compare_op=mybir.AluOpType.is_ge, fill=0.0,
            base=-124, pattern=[[-1, 128]], channel_multiplier=1)
    return bandP, bandN, bandL, bandLn


def build_module():
    nc = bacc.Bacc()
    x_dram = nc.dram_tensor("x", [IMG_PER_CORE, HW, HW], F32, kind="ExternalInput")
    y_dram = nc.dram_tensor("y", [IMG_PER_CORE, HW, HW], F32, kind="ExternalInput")
    out_dram = nc.dram_tensor("partials", [P, 1], F32, kind="ExternalOutput")

    with tile.TileContext(nc) as tc:
        with (
            tc.tile_pool(name="consts", bufs=1) as consts,
            tc.tile_pool(name="xin", bufs=8) as xpool,
            tc.tile_pool(name="yin", bufs=8) as ypool,
            tc.tile_pool(name="vt", bufs=8) as vtpool,
            tc.tile_pool(name="accp", bufs=1) as accpool,
            tc.tile_pool(name="vtps", bufs=2, space="PSUM") as vt_ps_pool,
            tc.tile_pool(name="hps", bufs=2, space="PSUM") as h_ps_pool,
        ):
            # Image 0's loads trace before the const-building: SWDGE emission
            # shares the Pool (gpsimd) engine with memset/affine_select, so
            # this starts HBM traffic ~2us earlier without promoting the
            # other DMAs above compute in scheduler priority.
            x0_sb = xpool.tile([P, KT, HW], F16, name="x_sb")
            y0_sb = ypool.tile([P, KT, HW], F16, name="y_sb")
            nc.gpsimd.dma_start(
                out=x0_sb, in_=x_dram[0].rearrange("(k p) w -> p k w", p=P))
            nc.gpsimd.dma_start(
                out=y0_sb, in_=y_dram[0].rearrange("(k p) w -> p k w", p=P))

            bandP, bandN, bandL, bandLn = _make_band_consts(nc, consts)
            # 5 abs-sum columns per image + 1 final column
            acc = accpool.tile([P, IMG_PER_CORE * 5], F32)
            nc.vector.memset(acc, 0.0)

            for i in range(IMG_PER_CORE):
                # ---- load + cast fp32 -> fp16 (SWDGE cast DMA) ----
                if i == 0:
                    x_sb, y_sb = x0_sb, y0_sb
                else:
                    x_sb = xpool.tile([P, KT, HW], F16, name="x_sb")
                    y_sb = ypool.tile([P, KT, HW], F16, name="y_sb")
                    nc.gpsimd.dma_start(
                        out=x_sb, in_=x_dram[i].rearrange("(k p) w -> p k w", p=P))
                    nc.gpsimd.dma_start(
                        out=y_sb, in_=y_dram[i].rearrange("(k p) w -> p k w", p=P))

                # ---- pass 1 + drain: VT[w-tile m] = (x - y) row-filtered ----
                vt_sb = vtpool.tile([P, KT, OUT], F16)
                for m in range(KT):
                    wb = slice(128 * m, 128 * (m + 1))
                    vt_ps = vt_ps_pool.tile([P, OUT], F32)
                    for src, band, bandc, isx in (
                        (x_sb, bandP, bandL, True),
                        (y_sb, bandN, bandLn, False),
                    ):
                        for k in range(KT):
                            o0 = 128 * k
                            n = 132 if k < 3 else 128
                            nc.tensor.matmul(
                                vt_ps[:, o0:o0 + n],
                                lhsT=src[:, k, wb],
                                rhs=band[:, 0:n],
                                start=(isx and k == 0),
                                stop=((not isx) and k == 3),
                            )
                        # o in [512, 516): contributions from rows 508..511
                        nc.tensor.matmul(
                            vt_ps[:, 512:516],
                            lhsT=src[:, 3, wb],
                            rhs=bandc[:, 0:4],
                            start=isx,
                            stop=not isx,
                        )
                    # drain PSUM fp32 -> SBUF fp16 on the ScalarEngine
                    nc.scalar.copy(out=vt_sb[:, m, :], in_=vt_ps)

                # ---- pass 2 + fused abs-reduce ----
                for m in range(5):
                    h_ps = h_ps_pool.tile([P, OUT], F32)
                    if m < 4:
                        pslice = slice(0, P)
                        parts = [(bandP[:, 0:128], m)]
                        if m >= 1:
                            parts.append((bandL, m - 1))
                    else:
                        pslice = slice(0, 4)
                        parts = [(bandL[:, 0:4], 3)]
                    for j, (lhsT, wsrc) in enumerate(parts):
                        first = j == 0
                        last = j == len(parts) - 1
                        nc.tensor.matmul(
                            h_ps[pslice, 0:512],
                            lhsT=lhsT,
                            rhs=vt_sb[:, wsrc, 0:512],
                            start=first,
                            stop=last,
                        )
                        nc.tensor.matmul(
                            h_ps[pslice, 512:516],
                            lhsT=lhsT,
                            rhs=vt_sb[:, wsrc, 512:516],
                            start=first,
                            stop=last,
                        )
                    col = i * 5 + m
                    nc.vector.tensor_reduce(
                        out=acc[pslice, col:col + 1],
                        in_=h_ps[pslice, :],
                        axis=mybir.AxisListType.X,
                        op=mybir.AluOpType.add,
                        apply_absolute_value=True,
                    )

            final = accpool.tile([P, 1], F32)
            nc.vector.tensor_reduce(
                out=final,
                in_=acc,
                axis=mybir.AxisListType.X,
                op=mybir.AluOpType.add,
            )
            nc.sync.dma_start(out=out_dram[:], in_=final)

    nc.finalize()
    return nc


_NC_CACHE = None


def kernel(x: np.ndarray, y: np.ndarray) -> np.ndarray:
    global _NC_CACHE
    if _NC_CACHE is None:
        _NC_CACHE = build_module()
    nc = _NC_CACHE

    x = np.ascontiguousarray(np.asarray(x, dtype=np.float32).reshape(64, HW, HW))
    y = np.ascontiguousarray(np.asarray(y, dtype=np.float32).reshape(64, HW, HW))

    in_maps = [
        {
            "x": x[IMG_PER_CORE * c:IMG_PER_CORE * (c + 1)],
            "y": y[IMG_PER_CORE * c:IMG_PER_CORE * (c + 1)],
        }
        for c in range(N_CORES)
    ]
    res = run_bass_kernel_spmd(nc, in_maps, core_ids=list(range(N_CORES)))
    total = np.float64(0.0)
    for r in res.results:
        total += r["partials"].astype(np.float64).sum()
    mean = total / (25.0 * 64 * OUT * OUT)
    return np.float32(mean)

